# revision 6
# baseline (speedup 1.0000x reference)
"""Trainium2 Bass kernel v2 for nn_E4_C4 (C4-equivariant involution CNN).

Contract: kernel(**inputs) takes FULL unsharded inputs and returns the FULL
output [8, 512, 32, 32] fp32. Data-parallel: 1 batch element per core.

v2 restructure vs baseline:
  - The dynamic weight map wmap[(g,p), q] is computed with 4 tap-batched
    GEMMs per rotation (M=128 rows = 2 kernel-rows x 7 taps x 8 groups,
    N=1024) instead of one replicated GEMM per tap: 16K PE cycles instead
    of 200K.
  - The 8->128 channel replication of wmap rows runs as per-tap K=64
    selection matmuls (sel matrix picks a tap column and replicates each
    group row to its 16 channel partitions).
  - Products multiply straight out of the replication PSUM on the DVE
    (all-dve KRN_MODES default): removing the ACT-evict hop from every
    tap's dependency chain measured faster than any 2x-DVE mix.
  - The Pool engine pre-sums KRN_POOL_PAIRS=2 pairs of product columns
    per kernel row (SBUF bf16), so the PE identity-accumulation runs 10
    instead of 14 matmuls per row.
  - Software pipelining: each row's identity-matmuls are emitted after
    the NEXT row's replication matmuls (lag-1), keeping the PE fed with
    ready work while the DVE produces the current row; prod pool bufs=4
    to cover the extra row of product lifetime.
  (A stride-0-source replication DMA and an indirect-DMA gather were
  prototyped and are cheaper on paper, but the descriptor lowering drops
  stride-0 dims and indirect DMA faults under this runtime, so the PE
  carries the replication.)
"""

import math
import os
from contextlib import ExitStack

import numpy as np

import concourse.bacc as bacc
import concourse.bass as bass
import concourse.tile as tile
from concourse import mybir
from concourse.bass_utils import run_bass_kernel_spmd

# ---- problem constants (hardcoded per contract) ----
B = 8
CIN = 128
COUT = 128
KK = 7
R = 2
G = 8
GC = 16
H = W = 32
S = H * W  # 1024
EPS = 1e-5
NCORES = 8
F32 = mybir.dt.float32
F32R_G = mybir.dt.float32r
BF16 = mybir.dt.bfloat16

# knobs
# dx positions whose product runs on Pool (gpsimd) instead of DVE
POOL_DX = tuple(
    int(t) for t in os.environ.get("KRN_POOL_DX", "3").split(",") if t != ""
)
# last PE_TAIL dx positions per row are replicated via PE rep8-matmul + ACT
# evict; the first 7-PE_TAIL via the gather DMA
PE_TAIL = int(os.environ.get("KRN_PE_TAIL", "7"))
# per-dx product path: 'act' = ACT evicts replicated PSUM to SBUF bf16 then
# DVE 2x product; 'dve'/'pool' = product reads the PSUM tile directly (1x)
MODES = os.environ.get("KRN_MODES", "dve,dve,dve,dve,dve,dve,dve").split(",")
WREP_BUFS = int(os.environ.get("KRN_WREP_BUFS", "3"))
PROD_BUFS = int(os.environ.get("KRN_PROD_BUFS", "4"))
# timing ablations: "" = full kernel; "pefloor" = idents read an always-ready
# tile (products/evicts skipped); "noident" = single dummy ident per (r, nh)
ABLATE = os.environ.get("KRN_ABLATE", "")
# pairs of product columns pre-summed on the (otherwise idle) Pool engine,
# halving those taps' PE identity-matmul accumulation: 0..3 pairs per row
POOL_PAIRS = int(os.environ.get("KRN_POOL_PAIRS", "2"))


# ------------------------------------------------------------------ host prep
def _c4_lift_np(w):
    Wr = np.stack([np.roll(w, r, axis=-1) for r in range(4)], axis=1)  # [o,4,i,4]
    o, _, i, _ = Wr.shape
    return Wr.reshape(o * 4, i * 4)


def _host_prep(v_w, c1_w, gn_g, gn_b, c2_w, c2_b):
    W1 = _c4_lift_np(np.asarray(c1_w, np.float32))  # [256, 512], rows c*4+r
    W1_r = W1.reshape(64, 4, 512).transpose(1, 0, 2).reshape(256, 512)
    W1T = np.ascontiguousarray(W1_r.T)  # [512, 256]

    Wv = _c4_lift_np(np.asarray(v_w, np.float32))  # [512, 512]
    Wv_r = Wv.reshape(128, 4, 512).transpose(1, 0, 2).reshape(512, 512)
    WvT = np.ascontiguousarray(Wv_r.T)  # [512, 512]

    gam_r = np.ascontiguousarray(
        np.asarray(gn_g, np.float32).reshape(64, 4).T.reshape(2, 128).T
    )  # [128, 2]
    bet_r = np.ascontiguousarray(
        np.asarray(gn_b, np.float32).reshape(64, 4).T.reshape(2, 128).T
    )

    c2_w = np.asarray(c2_w, np.float32)  # [392, 64], row g*49 + p
    c2_b = np.asarray(c2_b, np.float32)
    # c2t[kb+k, slot, mt, j]: lhsT for the tap-batched wmap GEMM of rotation
    # r = 2*slot + kb//64. Output row j = (dy%2)*64 + dx*8 + g in M-tile
    # mt = dy//2 holds Sum_k c2_w[g*49 + perm_r[dy*7+dx], k] * t1[kb+k, slot].
    c2t = np.zeros((128, 2, 4, 128), np.float32)
    wbias = np.zeros((128, 4, 4), np.float32)
    for r in range(4):
        kb = 64 * (r % 2)
        slot = r // 2
        perm = np.rot90(np.arange(49).reshape(7, 7), k=r).flatten()
        for dy in range(7):
            mt = dy // 2
            half = dy % 2
            for dx in range(7):
                row = perm[dy * 7 + dx]
                for g in range(8):
                    j = half * 64 + dx * 8 + g
                    c2t[kb : kb + 64, slot, mt, j] = c2_w[g * 49 + row, :]
                    wbias[j, r, mt] = c2_b[g * 49 + row]

    i128 = np.eye(128, dtype=np.float32)
    # sel[p, dx, j] = 1 iff p%64 == dx*8 + j//16: reading 64 wmap rows at
    # base partition 0/64, selects tap column dx and replicates each group
    # row to its 16 channel partitions.
    rep8 = np.zeros((128, 7, 128), np.float32)
    for p in range(128):
        for dx in range(7):
            if (p % 64) // 8 == dx:
                g = p % 8
                rep8[p, dx, g * 16 : (g + 1) * 16] = 1.0
    gmat = np.zeros((128, 64), np.float32)
    gmat[np.arange(128), np.arange(128) % 64] = 0.25
    emat = np.zeros((64, 128), np.float32)
    emat[np.arange(128) % 64, np.arange(128)] = 1.0
    # gather indices: for wrep partition p = g*16+c and tap column dx, read
    # bounced-wmap row dx*8 + g (per-(r,dy) row base goes in element_offset)
    p_idx = np.arange(128)
    gidx = (np.arange(7)[None, :] * 8 + (p_idx[:, None] // 16)).astype(np.int32)
    return W1T, WvT, gam_r, bet_r, c2t, wbias, i128, rep8, gmat, emat, gidx


# ------------------------------------------------------------------ bass build
def _build_module(loop_n=1):
    nc = bacc.Bacc(None)

    x_d = nc.dram_tensor("x", [512, S], F32R_G, kind="ExternalInput")
    w1t_d = nc.dram_tensor("w1t", [512, 256], F32R_G, kind="ExternalInput")
    wvt_d = nc.dram_tensor("wvt", [512, 512], F32R_G, kind="ExternalInput")
    c2t_d = nc.dram_tensor("c2t", [128, 2, 4, 128], BF16, kind="ExternalInput")
    wb_d = nc.dram_tensor("wbias", [128, 4, 4], F32, kind="ExternalInput")
    gam_d = nc.dram_tensor("gam", [128, 2], F32, kind="ExternalInput")
    bet_d = nc.dram_tensor("bet", [128, 2], F32, kind="ExternalInput")
    i128_d = nc.dram_tensor("i128", [128, 128], BF16, kind="ExternalInput")
    rep8_d = nc.dram_tensor("rep8", [128, 7, 128], BF16, kind="ExternalInput")
    gm_d = nc.dram_tensor("gmat", [128, 64], F32, kind="ExternalInput")
    em_d = nc.dram_tensor("emat", [64, 128], F32, kind="ExternalInput")
    gidx_d = nc.dram_tensor("gidx", [128, 7], mybir.dt.int32, kind="ExternalInput")
    # bounce buffer for the dynamic weight map: row (r*7+dy)*56 + dx*8 + g
    wmapD = nc.dram_tensor("wmapD", [4 * 7 * 56, S], BF16, kind="Internal")
    out_d = nc.dram_tensor("out", [512, S], F32, kind="ExternalOutput")

    AL = mybir.AluOpType
    AF = mybir.ActivationFunctionType

    with tile.TileContext(nc) as tc, ExitStack() as ctx:
        if loop_n > 1:
            ctx.enter_context(tc.For_i(0, loop_n, 1))
        consts = ctx.enter_context(tc.tile_pool(name="consts", bufs=1))
        sb = ctx.enter_context(tc.tile_pool(name="sb", bufs=1))
        small = ctx.enter_context(tc.tile_pool(name="small", bufs=8))
        wreps = ctx.enter_context(tc.tile_pool(name="wreps", bufs=WREP_BUFS))
        prods = ctx.enter_context(tc.tile_pool(name="prods", bufs=PROD_BUFS))
        outs = ctx.enter_context(tc.tile_pool(name="outs", bufs=2))
        phase1_psum = tc.tile_pool(name="psA", bufs=2, space="PSUM")
        psA = phase1_psum.__enter__()
        stat_psum = tc.tile_pool(name="psStat", bufs=1, space="PSUM")
        psStat = stat_psum.__enter__()

        # ---- load weights/constants into SBUF
        x_sb = sb.tile([128, 4, S], F32R_G)
        w1t_sb = sb.tile([128, 4, 256], F32R_G)
        wvt_sb = sb.tile([128, 4, 512], F32R_G)
        for kt in range(4):
            nc.sync.dma_start(out=x_sb[:, kt, :], in_=x_d[kt * 128 : (kt + 1) * 128, :])
            nc.sync.dma_start(
                out=w1t_sb[:, kt, :], in_=w1t_d[kt * 128 : (kt + 1) * 128, :]
            )
            nc.sync.dma_start(
                out=wvt_sb[:, kt, :], in_=wvt_d[kt * 128 : (kt + 1) * 128, :]
            )
        c2t_sb = sb.tile([128, 2, 4, 128], BF16)
        nc.sync.dma_start(out=c2t_sb, in_=c2t_d[:])
        wb_sb = consts.tile([128, 4, 4], F32)
        nc.sync.dma_start(out=wb_sb, in_=wb_d[:])
        gam_sb = consts.tile([128, 2], F32)
        nc.sync.dma_start(out=gam_sb, in_=gam_d[:])
        bet_sb = consts.tile([128, 2], F32)
        nc.sync.dma_start(out=bet_sb, in_=bet_d[:])
        i128_sb = consts.tile([128, 128], BF16)
        nc.sync.dma_start(out=i128_sb, in_=i128_d[:])
        rep8_sb = consts.tile([128, 7, 128], BF16)
        nc.sync.dma_start(out=rep8_sb, in_=rep8_d[:])
        gm_sb = consts.tile([128, 64], F32)
        nc.sync.dma_start(out=gm_sb, in_=gm_d[:])
        em_sb = consts.tile([64, 128], F32)
        nc.sync.dma_start(out=em_sb, in_=em_d[:])
        gidx_sb = consts.tile([128, 7], mybir.dt.int32)
        nc.sync.dma_start(out=gidx_sb, in_=gidx_d[:])

        eps_t = consts.tile([64, 1], F32)
        nc.vector.memset(eps_t, EPS)

        # warm the ACT function tables under the DMA shadow
        warm = consts.tile([1, 1], F32)
        nc.vector.memset(warm, 1.0)
        nc.scalar.activation(out=warm, in_=warm, func=AF.Relu)
        nc.scalar.activation(out=warm, in_=warm, func=AF.Sqrt)
        nc.scalar.activation(out=warm, in_=warm, func=AF.Identity)

        # zero-padded v, bf16, pixels on the free dim: [128, r, 38*38]
        vpad = sb.tile([128, 4, 38 * 38], BF16)
        nc.gpsimd.memset(vpad.bitcast(mybir.dt.uint16), 0)

        # ---- GEMM1: t [256, 1024]; both M-tiles stay in PSUM through GN
        ps_t = []
        for mt in range(2):
            pt = psA.tile([128, S], F32, tag="mm_out")
            for nh in range(2):
                for kt in range(4):
                    nc.tensor.matmul(
                        pt[:, nh * 512 : (nh + 1) * 512],
                        lhsT=w1t_sb[:, kt, mt * 128 : (mt + 1) * 128],
                        rhs=x_sb[:, kt, nh * 512 : (nh + 1) * 512],
                        start=(kt == 0),
                        stop=(kt == 3),
                    )
            ps_t.append(pt)

        # ---- GroupNorm stats (read PSUM directly)
        stats = []
        for t in range(2):
            st6 = small.tile([128, 2, 6], F32, tag="st6")
            for hh in range(2):
                nc.vector.bn_stats(
                    out=st6[:, hh, :], in_=ps_t[t][:, hh * 512 : (hh + 1) * 512]
                )
            mv = small.tile([128, 2], F32, tag="mv")
            nc.vector.bn_aggr(out=mv, in_=st6)
            nc.vector.scalar_tensor_tensor(
                out=mv[:, 1:2],
                in0=mv[:, 0:1],
                scalar=mv[:, 0:1],
                in1=mv[:, 1:2],
                op0=AL.mult,
                op1=AL.add,
            )
            stats.append(mv)

        ps_g = psStat.tile([64, 2], F32, tag="gstat")
        for t in range(2):
            nc.tensor.matmul(
                ps_g, lhsT=gm_sb, rhs=stats[t], start=(t == 0), stop=(t == 1)
            )
        gss = small.tile([64, 2], F32, tag="gss")
        nc.vector.tensor_copy(out=gss, in_=ps_g)
        gmv = small.tile([64, 2], F32, tag="gmv")  # [mean_g, rstd_g]
        nc.vector.tensor_copy(out=gmv[:, 0:1], in_=gss[:, 0:1])
        gv = small.tile([64, 1], F32, tag="gv")
        nc.vector.tensor_mul(out=gv, in0=gss[:, 0:1], in1=gss[:, 0:1])
        nc.vector.tensor_sub(out=gv, in0=gss[:, 1:2], in1=gv)
        nc.scalar.activation(out=gv, in_=gv, func=AF.Sqrt, bias=eps_t, scale=1.0)
        nc.vector.reciprocal(out=gmv[:, 1:2], in_=gv)

        ps_e = psStat.tile([128, 2], F32, tag="gstat")
        nc.tensor.matmul(ps_e, lhsT=em_sb, rhs=gmv, start=True, stop=True)

        # per-partition scale/bias; apply GN + ReLU into t1 (bf16)
        t1_sb = sb.tile([128, 2, S], BF16)
        scb = small.tile([128, 2, 2], F32, tag="scb")
        for t in range(2):
            nc.vector.tensor_mul(
                out=scb[:, t, 0:1], in0=ps_e[:, 1:2], in1=gam_sb[:, t : t + 1]
            )
            nc.vector.tensor_mul(out=scb[:, t, 1:2], in0=ps_e[:, 0:1], in1=scb[:, t, 0:1])
            nc.vector.tensor_sub(
                out=scb[:, t, 1:2], in0=bet_sb[:, t : t + 1], in1=scb[:, t, 1:2]
            )
            nc.scalar.activation(
                out=t1_sb[:, t, :],
                in_=ps_t[t][:, :],
                func=AF.Relu,
                scale=scb[:, t, 0:1],
                bias=scb[:, t, 1:2],
            )

        # ---- GEMMv for all 4 rotations -> vpad (bf16)
        for r in range(4):
            ps_v = psA.tile([128, S], F32, tag="mm_out")
            for nh in range(2):
                for kt in range(4):
                    nc.tensor.matmul(
                        ps_v[:, nh * 512 : (nh + 1) * 512],
                        lhsT=wvt_sb[:, kt, r * 128 : (r + 1) * 128],
                        rhs=x_sb[:, kt, nh * 512 : (nh + 1) * 512],
                        start=(kt == 0),
                        stop=(kt == 3),
                    )
            vpad_int = vpad[:, r, :].rearrange("q (yy xx) -> q yy xx", xx=38)[
                :, 3:35, 3:35
            ]
            nc.scalar.activation(
                out=vpad_int,
                in_=ps_v.rearrange("q (y x) -> q y x", x=32),
                func=AF.Identity,
            )

        # phase-1 PSUM pools close here; the involution reuses their banks
        stat_psum.__exit__(None, None, None)
        phase1_psum.__exit__(None, None, None)
        psW = ctx.enter_context(tc.tile_pool(name="psW", bufs=3, space="PSUM"))
        psO = ctx.enter_context(tc.tile_pool(name="psO", bufs=1, space="PSUM"))

        # ---- tap-batched wmap GEMMs for all (r, mt), evicted to SBUF bf16,
        # then bounced per kernel-row to DRAM for the gather-replication
        wmap_sb = sb.tile([128, 4, 4, S], BF16)  # [j, r, mt, q]
        for r in range(4):
            kb = 64 * (r % 2)
            slot = r // 2
            for mt in range(4):
                w_ps = psW.tile([128, S], F32, tag="wmap")
                for nh in range(2):
                    nc.tensor.matmul(
                        w_ps[:, nh * 512 : (nh + 1) * 512],
                        lhsT=c2t_sb[kb : kb + 64, slot, mt, :],
                        rhs=t1_sb[kb : kb + 64, slot, nh * 512 : (nh + 1) * 512],
                        start=True,
                        stop=True,
                    )
                nc.scalar.activation(
                    out=wmap_sb[:, r, mt, :],
                    in_=w_ps,
                    func=AF.Identity,
                    bias=wb_sb[:, r, mt : mt + 1],
                    scale=1.0,
                )
                if PE_TAIL < 7:
                    for half in range(2):
                        dy = 2 * mt + half
                        if dy > 6:
                            continue
                        row0 = (r * 7 + dy) * 56
                        nc.sync.dma_start(
                            out=wmapD[row0 : row0 + 56, :],
                            in_=wmap_sb[half * 64 : half * 64 + 56, r, mt, :],
                        )

        # ---- involution: per (r, kernel-row dy): replicate -> product -> PE acc
        n_gather = 7 - PE_TAIL

        for r in range(4):
            out_ps = psO.tile([128, S], F32, tag="out_ps")
            vp = vpad[:, r, :]

            def emit_idents(prod, dy, skip, out_ps=out_ps):
                first_dx = min(dx for dx in range(7) if dx not in skip)
                for nh in range(2):
                    for dx in range(7):
                        if dx in skip:
                            continue
                        if ABLATE == "noident" and not (dx == 6):
                            continue
                        nc.tensor.matmul(
                            out_ps[:, nh * 512 : (nh + 1) * 512],
                            lhsT=i128_sb,
                            rhs=prod[:, dx, nh * 512 : (nh + 1) * 512],
                            start=(
                                dy == 0 and (dx == first_dx or ABLATE == "noident")
                            ),
                            stop=(dy == 6 and dx == 6),
                        )

            pend = None
            for dy in range(7):
                mt = dy // 2
                half = dy % 2
                need_wrep = n_gather > 0 or any(
                    m != "dve" for m in MODES[n_gather:]
                )
                wrep = (
                    wreps.tile([128, 7, S], BF16, tag="wrep") if need_wrep else None
                )
                prod = prods.tile([128, 7, S], BF16, tag="prod")
                prod4 = prod.rearrange("q t (y x) -> q t y x", x=32)
                wrep4 = (
                    wrep.rearrange("q t (y x) -> q t y x", x=32) if need_wrep else None
                )
                if n_gather > 0:
                    nc.gpsimd.indirect_dma_start(
                        out=wrep[:, :n_gather, :],
                        out_offset=None,
                        in_=wmapD[:],
                        in_offset=bass.IndirectOffsetOnAxis(
                            ap=gidx_sb[:, :n_gather], axis=0
                        ),
                        element_offset=(r * 7 + dy) * 56 * S,
                    )
                hb = half * 64
                act_dxs = []
                if ABLATE == "pefloor":
                    for dx in range(7):
                        rp = psW.tile([128, S], F32, tag="wmap")
                        for nh in range(2):
                            nc.tensor.matmul(
                                rp[:, nh * 512 : (nh + 1) * 512],
                                lhsT=rep8_sb[hb : hb + 64, dx, :],
                                rhs=wmap_sb[hb : hb + 64, r, mt, nh * 512 : (nh + 1) * 512],
                                start=True,
                                stop=True,
                            )
                    for nh in range(2):
                        for dx in range(7):
                            nc.tensor.matmul(
                                out_ps[:, nh * 512 : (nh + 1) * 512],
                                lhsT=i128_sb,
                                rhs=wmap_sb[:, r, mt, nh * 512 : (nh + 1) * 512],
                                start=(dy == 0 and dx == 0),
                                stop=(dy == 6 and dx == 6),
                            )
                    continue
                for dx in range(7):
                    mode = MODES[dx] if dx >= n_gather else "gather"
                    if mode == "gather":
                        continue
                    rp = psW.tile([128, S], F32, tag="wmap")
                    for nh in range(2):
                        nc.tensor.matmul(
                            rp[:, nh * 512 : (nh + 1) * 512],
                            lhsT=rep8_sb[hb : hb + 64, dx, :],
                            rhs=wmap_sb[hb : hb + 64, r, mt, nh * 512 : (nh + 1) * 512],
                            start=True,
                            stop=True,
                        )
                    vap = bass.AP(
                        tensor=vp.tensor,
                        offset=vp.offset + dy * 38 + dx,
                        ap=[list(vp.ap[0]), [38, 32], [1, 32]],
                    )
                    if mode == "act":
                        nc.scalar.activation(
                            out=wrep[:, dx, :], in_=rp, func=AF.Identity
                        )
                        act_dxs.append(dx)
                    elif mode == "dve":
                        nc.vector.tensor_mul(
                            out=prod4[:, dx, :, :],
                            in0=rp.rearrange("q (y x) -> q y x", x=32),
                            in1=vap,
                        )
                    else:  # pool: ACT evict, then Pool product from SBUF
                        nc.scalar.activation(
                            out=wrep[:, dx, :], in_=rp, func=AF.Identity
                        )
                        nc.gpsimd.tensor_mul(
                            out=prod4[:, dx, :, :],
                            in0=wrep4[:, dx, :, :],
                            in1=vap,
                        )
                # DVE 2x products for gather/act taps (bf16 SBUF operands)
                sb_dxs = list(range(n_gather)) + act_dxs
                runs = []
                for dx in sb_dxs:
                    if runs and runs[-1][0] + runs[-1][1] == dx:
                        runs[-1][1] += 1
                    else:
                        runs.append([dx, 1])
                for nh in range(2):
                    for dx0, glen in runs:
                        base = vp.offset + (nh * 16 + dy) * 38 + dx0
                        nc.vector.tensor_mul(
                            out=prod4[:, dx0 : dx0 + glen, nh * 16 : (nh + 1) * 16, :],
                            in0=wrep4[:, dx0 : dx0 + glen, nh * 16 : (nh + 1) * 16, :],
                            in1=bass.AP(
                                tensor=vp.tensor,
                                offset=base,
                                ap=[list(vp.ap[0]), [1, glen], [38, 16], [1, 32]],
                            ),
                        )
                # Pool pre-adds: fold prod pairs so the PE skips those idents
                pairs = [(0, 1), (2, 3), (4, 5)][:POOL_PAIRS]
                skip = {a for a, _ in pairs}
                for a, b in pairs:
                    nc.gpsimd.tensor_add(
                        out=prod[:, b, :], in0=prod[:, a, :], in1=prod[:, b, :]
                    )
                # software pipeline: emit the PREVIOUS row's identity-matmuls
                # here, after this row's replication matmuls, so the PE has
                # ready work while the DVE produces this row's products
                if pend is not None:
                    emit_idents(*pend)
                pend = (prod, dy, skip)

            if pend is not None:
                emit_idents(*pend)
            out_sb = outs.tile([128, S], F32, tag="out_sb")
            nc.scalar.activation(out=out_sb, in_=out_ps, func=AF.Identity)
            out_view = out_d[:].rearrange("(o r) s -> r o s", r=4)[r]
            nc.sync.dma_start(out=out_view, in_=out_sb)

    nc.compile()
    return nc


_CACHED = {}


def _get_module(loop_n=1):
    key = f"nc{loop_n}"
    if key not in _CACHED:
        _CACHED[key] = _build_module(loop_n)
    return _CACHED[key]


# ------------------------------------------------------------------ entrypoint
def _prepare_in_maps(x, v_w, c1_w, gn_g, gn_b, c2_w, c2_b):
    import ml_dtypes

    x = np.ascontiguousarray(np.asarray(x, np.float32))
    (W1T, WvT, gam_r, bet_r, c2t, wbias, i128, rep8, gmat, emat, gidx) = _host_prep(
        v_w, c1_w, gn_g, gn_b, c2_w, c2_b
    )
    shared = {
        "w1t": W1T,
        "wvt": WvT,
        "c2t": c2t.astype(ml_dtypes.bfloat16),
        "wbias": wbias,
        "gam": gam_r,
        "bet": bet_r,
        "i128": i128.astype(ml_dtypes.bfloat16),
        "rep8": rep8.astype(ml_dtypes.bfloat16),
        "gmat": gmat,
        "emat": emat,
        "gidx": gidx,
    }
    in_maps = []
    for c in range(NCORES):
        m = dict(shared)
        m["x"] = np.ascontiguousarray(x[c].reshape(512, S))
        in_maps.append(m)
    return in_maps, {}


def kernel(x, v_w, c1_w, gn_g, gn_b, c2_w, c2_b):
    in_maps, build_kwargs = _prepare_in_maps(x, v_w, c1_w, gn_g, gn_b, c2_w, c2_b)
    nc = _get_module(**build_kwargs)
    res = run_bass_kernel_spmd(nc, in_maps, core_ids=list(range(NCORES)))
    _CACHED["last_results"] = res
    out = np.stack([res.results[c]["out"] for c in range(NCORES)])
    return out.reshape(B, 512, H, W)
